# revision 6
# baseline (speedup 1.0000x reference)
"""Bass/Tile Trainium2 kernel for additive (Bahdanau/'cat') attention.

Problem (per batch b):
  A[i,d]      = sum_a context[i,a] * attn_w[a,d] + attn_b[d]
  O[o,d]      = sum_e output[o,e]  * dec_w[e,d]  + dec_b[d]
  scores[o,i] = sum_d query_w[d] * tanh(A[i,d] + O[o,d])   (+query_b: softmax-invariant)
  attn        = softmax_i(scores)
  mix[o,a]    = sum_i attn[o,i] * context[i,a]
  out[o,d]    = tanh([mix | output] @ out_w + out_b)

Sharding: pure data-parallel over batch, B=8 -> one batch per NeuronCore,
weights broadcast, no collectives.

Per-core structure (v2 — engine-balanced rewrite):
  * The 16.7M-element nonlinearity is split across three engines per
    (o, d-chunk) unit: rows j < JT[dc] of each 16-o group go through the
    tanh lane (broadcast-add on DVE/gpsimd + batched ACT tanh); rows
    j >= JT[dc] go through an exp-domain lane computed entirely on the DVE
    by ONE fused custom op per block:
        R' = 1/(1 + e^{2A} * e^{2O})   (tanh(x) = 1 - 2*R')
    implemented as u = Src0*Src1 + 1, bitwise-not seed, one Newton pass
    (7 ALU stages, max rel err 1.7e-4 * ... ~0.2%), batched over the o's of
    the block with stride-0 broadcast APs.  The PE reduction weight for exp
    rows is -2*q_d (vs q_d for tanh rows); the leftover per-row constant
    sum q_d is softmax-invariant.
  * Weights/context are pre-cast to bf16 on the host (layout prep):
    no on-device cast traffic and half the DMA bytes.
  * q-reduction on the PE with zero-padded stationaries: group g of 16 o's
    accumulates into PSUM partitions 32g..32g+15 via tile_position=(0,32g),
    so adjacent groups' matmuls run concurrently in different column groups
    of the PE array and every PSUM->SBUF evacuation starts at a legal
    32-partition boundary (no scatter DMAs).
  * Scores stay in PSUM; softmax exp reads PSUM directly (ACT, accum_out
    for the normalizer).  attn^T columns are packed per group, the mix and
    the final projection run per phase-half (o 0..31 / 32..63) so the first
    half's epilogue + projection hide under the second half's main loop.
"""

import numpy as np
import ml_dtypes

import concourse.bass as bass
import concourse.tile as tile
import concourse.bass_utils as bass_utils
from concourse import bacc, mybir, dve_ops
from concourse.dve_ops import DveOp
from concourse.dve_spec import Spec, Src0, Src1, AluOp, Bin, One, C0, C1
from concourse.masks import make_identity

B, OUT_LEN, IN_LEN, DEC, ATTN = 8, 64, 512, 512, 512
P = 128
F32 = mybir.dt.float32
BF16 = mybir.dt.bfloat16
AF = mybir.ActivationFunctionType

G = 16                    # o's per group (= PSUM col-group rows)
NG = OUT_LEN // G         # 4 groups
DC = DEC // P             # 4 d-chunks
AC = ATTN // P            # 4 a-chunks
IC = IN_LEN // P          # 4 i-chunks
EC = DEC // P             # 4 e-chunks
CC = (ATTN + DEC) // P    # 8 combined chunks

N_CORES = 8

# tanh rows per (group, dc); rows JT..16 go to the DVE exp lane.
JT = (10, 10, 10, 10)
# engine for each tanh-lane add: 'G' = gpsimd, 'D' = DVE.  ~3/4 gpsimd.
ADD_PAT = ("G", "G", "G", "D", "G", "G", "D", "G", "G", "D", "G", "D", "G", "G", "D", "G")

# seed + one-Newton reciprocal constants (Chebyshev pair for the
# [-4.5, -4] interval x*bitcast(~x) lands in; same as RECIP_APPROX_FAST).
C0V, C1V = -0.23549792, 2.0017324

# ---- fused DVE op: out = 1/(Src0*Src1 + 1) ---------------------------------
_u = Src0 * Src1 + One
_nu = Bin(AluOp.BITWISE_NOT, _u, _u)
_y0 = _nu * C0
_RECIP_BODY = _y0 * (C1 - _u * _y0)


def _recip_ref(in0, in1, c0, c1, c2):
    u = (in0.astype(np.float32) * in1.astype(np.float32) + np.float32(1.0)).astype(
        np.float32
    )
    nx = (~u.view(np.int32)).view(np.float32)
    y0 = nx * np.float32(c0)
    return (y0 * (np.float32(c1) - u * y0)).astype(np.float32)


RECIP_AFFINE1 = DveOp(
    "RECIP_AFFINE1_ANT",
    Spec(body=_RECIP_BODY, reference=_recip_ref),
    subdim=False,
    uops_sha={"v3": "4a6026d53837a2bc", "v4": "9de0b962752db8fb"},
)

if RECIP_AFFINE1.name not in dve_ops._SUB_OPCODE_FOR_NAME:
    dve_ops.OPS.append(RECIP_AFFINE1)
    dve_ops.CUSTOM_DVE_SPECS[RECIP_AFFINE1.name] = RECIP_AFFINE1.spec
    dve_ops._SUB_OPCODE_FOR_NAME[RECIP_AFFINE1.name] = (
        dve_ops._CUSTOM_DVE_ROW_BASE + len(dve_ops.OPS) - 1
    )


def _emit_block(nc, g, dc, epool, fpool, ATb, OTb, Pexp, Qexp, QZ, ps, split_tanh):
    """One (group, d-chunk) unit block: 16 o's, one d-chunk of 128."""
    o0 = G * g
    jt = JT[dc]
    ne = G - jt
    E = epool.tile([P, jt, IN_LEN], BF16, tag="E", name=f"E_{g}_{dc}")
    Fc = fpool.tile([P, G, IN_LEN], BF16, tag="F", name=f"F_{g}_{dc}")
    for j in range(jt):
        eng = nc.gpsimd if ADD_PAT[j] == "G" else nc.vector
        eng.tensor_scalar_add(E[:, j, :], ATb[:, dc, :], OTb[:, dc, o0 + j : o0 + j + 1])
    # exp lane: one fused op for rows jt..16
    if ne:
        nc.vector._custom_dve(
            RECIP_AFFINE1,
            out=Fc[:, jt:G, :],
            in0=Pexp[:, dc, :].unsqueeze(1).broadcast_to([P, ne, IN_LEN]),
            in1=Qexp[:, dc, o0 + jt : o0 + G].unsqueeze(2).broadcast_to([P, ne, IN_LEN]),
            s0=C0V,
            s1=C1V,
        )
    if split_tanh:
        h = jt // 2
        nc.scalar.activation(Fc[:, 0:h, :], E[:, 0:h, :], AF.Tanh)
        nc.scalar.activation(Fc[:, h:jt, :], E[:, h:jt, :], AF.Tanh)
    else:
        nc.scalar.activation(Fc[:, 0:jt, :], E[:, 0:jt, :], AF.Tanh)
    # q-reduction: exp rows first (ready earlier), then tanh rows
    pv = ps[32 * g : 32 * g + G, :]
    order = list(range(jt, G)) + list(range(jt))
    for k, j in enumerate(order):
        nc.tensor.matmul(
            pv,
            QZ[:, dc, j, :],
            Fc[:, j, :],
            start=(dc == 0 and k == 0),
            stop=(dc == DC - 1 and k == G - 1),
            tile_position=(0, 32 * g),
            skip_group_check=True,
        )


def _epilogue_phase(nc, gs, ps, ident_bf, exp_sb, sums, recip, attn_sb, attn_bf,
                    attnT_bf, ctx_bf, combT_bf, psum, attn_d):
    """softmax + attn^T + mix for groups gs (one phase half)."""
    for g in gs:
        sl = slice(32 * g, 32 * g + G)
        nc.scalar.activation(exp_sb[sl, :], ps[sl, :], AF.Exp, accum_out=sums[sl, :])
        nc.vector.reciprocal(recip[sl, :], sums[sl, :])
        nc.vector.tensor_scalar_mul(attn_bf[sl, :], exp_sb[sl, :], recip[sl, :])
        nc.vector.tensor_scalar_mul(attn_sb[sl, :], exp_sb[sl, :], recip[sl, :])
        nc.sync.dma_start(attn_d[G * g : G * g + G, :], attn_sb[sl, :])
        for ic in range(IC):
            pt = psum.tile([P, G], BF16, tag="sm", bufs=2, name=f"pt_{g}_{ic}")
            nc.tensor.transpose(
                pt[:], attn_bf[sl, ic * P : (ic + 1) * P], ident_bf[sl, sl],
                tile_position=(32 * g, 0),
            )
            nc.vector.tensor_copy(attnT_bf[:, ic, G * g : G * g + G], pt[:])
    # mix^T for this phase's 32 o-columns
    c0 = G * gs[0]
    cols = slice(c0, c0 + 2 * G)
    for ac in range(AC):
        pm = psum.tile([P, 2 * G], F32, tag="sm", bufs=2, name=f"pm_{gs[0]}_{ac}")
        for ic in range(IC):
            nc.tensor.matmul(
                pm[:],
                ctx_bf[:, ic, ac * P : (ac + 1) * P],
                attnT_bf[:, ic, cols],
                start=(ic == 0),
                stop=(ic == IC - 1),
            )
        nc.vector.tensor_copy(combT_bf[:, ac, cols], pm[:])


def _project_half(nc, ph, combT_bf, out_w_bf, ones_bf, outb_row_bf, po_f, out_sb,
                  out_d):
    """final projection + tanh + store for o rows 32*ph..32*ph+31."""
    hsl = slice(32 * ph, 32 * ph + 32)
    pv = po_f[hsl, :]
    for k in range(CC):
        nc.tensor.matmul(
            pv,
            combT_bf[:, k, hsl],
            out_w_bf[:, k, :],
            start=(k == 0),
            stop=False,
            tile_position=(0, 32 * ph),
        )
    nc.tensor.matmul(
        pv, ones_bf[:, 0:32], outb_row_bf[:], start=False, stop=True,
        tile_position=(0, 32 * ph),
    )
    nc.scalar.activation(out_sb[hsl, :], pv, AF.Tanh)
    nc.sync.dma_start(out_d[hsl, :], out_sb[hsl, :])


def _build_body(tc):
    nc = tc.nc

    # ---- DRAM I/O (per-core shard shapes; weights pre-cast bf16 on host) ----
    ctxT_d = nc.dram_tensor("ctx_t_bf", [ATTN, IN_LEN], BF16, kind="ExternalInput").ap()
    ctx_d = nc.dram_tensor("ctx_bf", [IN_LEN, ATTN], BF16, kind="ExternalInput").ap()
    outT_d = nc.dram_tensor("out_t_bf", [DEC, OUT_LEN], BF16, kind="ExternalInput").ap()
    attn_w_d = nc.dram_tensor("attn_w_bf", [ATTN, DEC], BF16, kind="ExternalInput").ap()
    dec_w_d = nc.dram_tensor("dec_w_bf", [DEC, DEC], BF16, kind="ExternalInput").ap()
    out_w_d = nc.dram_tensor("out_w_bf", [ATTN + DEC, DEC], BF16, kind="ExternalInput").ap()
    outb_row_d = nc.dram_tensor("outb_row_bf", [1, DEC], BF16, kind="ExternalInput").ap()
    ab_d = nc.dram_tensor("ab", [P, DC], F32, kind="ExternalInput").ap()
    ab2_d = nc.dram_tensor("ab2", [P, DC], F32, kind="ExternalInput").ap()
    db_d = nc.dram_tensor("db", [P, DC], F32, kind="ExternalInput").ap()
    db2_d = nc.dram_tensor("db2", [P, DC], F32, kind="ExternalInput").ap()
    qz_d = nc.dram_tensor("qz", [P, DC * G * G], BF16, kind="ExternalInput").ap()
    out_d = nc.dram_tensor("out", [OUT_LEN, DEC], F32, kind="ExternalOutput").ap()
    attn_d = nc.dram_tensor("attn", [OUT_LEN, IN_LEN], F32, kind="ExternalOutput").ap()

    from contextlib import ExitStack

    with ExitStack() as ctx:
        const = ctx.enter_context(tc.tile_pool(name="const", bufs=1))
        statics = ctx.enter_context(tc.tile_pool(name="statics", bufs=1))
        epool = ctx.enter_context(tc.tile_pool(name="epool", bufs=3))
        fpool = ctx.enter_context(tc.tile_pool(name="fpool", bufs=3))
        psum = ctx.enter_context(tc.tile_pool(name="psum", bufs=2, space="PSUM"))

        # ---------------- constants / identity / warmup ----------------
        ident = const.tile([P, P], F32)
        make_identity(nc, ident)
        ident_bf = const.tile([P, P], BF16)
        nc.vector.tensor_copy(ident_bf[:], ident[:])

        # HAM warmup: real matmul activity flips the PE clock gate to 8/8.
        wu = psum.tile([P, P], F32, tag="mm", bufs=2)
        for _ in range(10):
            nc.tensor.matmul(wu[:], ident_bf[:], ident_bf[:], start=True, stop=True)

        # ---------------- input DMAs ----------------
        ctxT_bf = statics.tile([P, AC, IN_LEN], BF16)    # [a%, ac, i]
        attn_w_bf = statics.tile([P, AC, DEC], BF16)     # [a%, ac, d]
        dec_w_bf = statics.tile([P, EC, DEC], BF16)      # [e%, ec, d]
        ctx_bf = statics.tile([P, IC, ATTN], BF16)       # [i%, ic, a]
        out_w_bf = statics.tile([P, CC, DEC], BF16)      # [c%, cc, d]
        combT_bf = statics.tile([P, CC, OUT_LEN], BF16)  # [c%, cc, o]
        QZ = const.tile([P, DC, G, G], BF16)
        ab = const.tile([P, DC], F32)
        ab2 = const.tile([P, DC], F32)
        db = const.tile([P, DC], F32)
        db2 = const.tile([P, DC], F32)
        outb_row_bf = const.tile([1, DEC], BF16)
        ones_bf = const.tile([1, OUT_LEN], BF16)
        nc.vector.memset(ones_bf[:], 1.0)

        for ac in range(AC):
            nc.sync.dma_start(ctxT_bf[:, ac, :], ctxT_d[ac * P : (ac + 1) * P, :])
        for ac in range(AC):
            nc.scalar.dma_start(attn_w_bf[:, ac, :], attn_w_d[ac * P : (ac + 1) * P, :])
        for ec in range(EC):
            nc.sync.dma_start(dec_w_bf[:, ec, :], dec_w_d[ec * P : (ec + 1) * P, :])
        for ec in range(EC):
            # output^T chunks double as combined^T chunks 4..7
            nc.scalar.dma_start(combT_bf[:, EC + ec, :], outT_d[ec * P : (ec + 1) * P, :])
        nc.scalar.dma_start(QZ[:], qz_d.rearrange("p (dc j m) -> p dc j m", dc=DC, j=G))
        for t, d in ((ab, ab_d), (ab2, ab2_d), (db, db_d), (db2, db2_d)):
            nc.scalar.dma_start(t[:], d)
        nc.scalar.dma_start(outb_row_bf[:], outb_row_d)
        # needed later (mix / final projection): land during phase A
        for ic in range(IC):
            nc.sync.dma_start(ctx_bf[:, ic, :], ctx_d[ic * P : (ic + 1) * P, :])
        for cc in range(CC):
            nc.sync.dma_start(out_w_bf[:, cc, :], out_w_d[cc * P : (cc + 1) * P, :])

        # ---------------- per-dc prologue: O^T, A^T, exps ----------------
        ATb = statics.tile([P, DC, IN_LEN], BF16)   # A^T + attn_b
        OTb = statics.tile([P, DC, OUT_LEN], F32)   # O^T + dec_b
        Pexp = statics.tile([P, DC, IN_LEN], BF16)  # e^{2(A+b)}
        Qexp = statics.tile([P, DC, OUT_LEN], F32)  # e^{2(O+b)}

        def _prologue_dc(dc):
            po = psum.tile([P, OUT_LEN], F32, tag="sm", bufs=2, name=f"po_{dc}")
            for ec in range(EC):
                nc.tensor.matmul(
                    po[:],
                    dec_w_bf[:, ec, dc * P : (dc + 1) * P],
                    combT_bf[:, EC + ec, :],
                    start=(ec == 0),
                    stop=(ec == EC - 1),
                )
            nc.vector.tensor_scalar_add(OTb[:, dc, :], po[:], db[:, dc : dc + 1])
            nc.scalar.activation(
                Qexp[:, dc, :], po[:], AF.Exp, bias=db2[:, dc : dc + 1], scale=2.0
            )
            pa = psum.tile([P, IN_LEN], F32, tag="mm", bufs=2, name=f"pa_{dc}")
            for ac in range(AC):
                nc.tensor.matmul(
                    pa[:],
                    attn_w_bf[:, ac, dc * P : (dc + 1) * P],
                    ctxT_bf[:, ac, :],
                    start=(ac == 0),
                    stop=(ac == AC - 1),
                )
            nc.vector.tensor_scalar_add(ATb[:, dc, :], pa[:], ab[:, dc : dc + 1])
            nc.scalar.activation(
                Pexp[:, dc, :], pa[:], AF.Exp, bias=ab2[:, dc : dc + 1], scale=2.0
            )

        # ---------------- epilogue tiles ----------------
        exp_sb = statics.tile([P, IN_LEN], F32)
        sums = statics.tile([P, 1], F32)
        recip = statics.tile([P, 1], F32)
        attn_sb = statics.tile([P, IN_LEN], F32)
        attn_bf = statics.tile([P, IN_LEN], BF16)
        attnT_bf = statics.tile([P, IC, OUT_LEN], BF16)
        out_sb = statics.tile([OUT_LEN, DEC], F32)
        psA = psum.tile([P, IN_LEN], F32, tag="sc", bufs=2, name="psA")
        psB = psum.tile([P, IN_LEN], F32, tag="sc", bufs=2, name="psB")
        po_f = psum.tile([OUT_LEN, DEC], F32, tag="pj", bufs=1, name="po_f")

        ep_args = (ident_bf, exp_sb, sums, recip, attn_sb, attn_bf,
                   attnT_bf, ctx_bf, combT_bf, psum, attn_d)
        blk = (epool, fpool, ATb, OTb, Pexp, Qexp, QZ)

        # ---------------- main loop ----------------
        _prologue_dc(0)
        for dc in range(DC):
            if dc + 1 < DC:
                _prologue_dc(dc + 1)
            _emit_block(nc, 0, dc, *blk, psA, split_tanh=(dc == 0))
            _emit_block(nc, 1, dc, *blk, psA, split_tanh=False)
        # phase B dc0 first so ACT keeps busy while phase A matmuls drain
        _emit_block(nc, 2, 0, *blk, psB, split_tanh=False)
        _emit_block(nc, 3, 0, *blk, psB, split_tanh=False)
        _epilogue_phase(nc, (0, 1), psA, *ep_args)
        for dc in range(1, DC):
            _emit_block(nc, 2, dc, *blk, psB, split_tanh=False)
            _emit_block(nc, 3, dc, *blk, psB, split_tanh=(dc == DC - 1))
        # half-A projection hides under phase B
        _project_half(nc, 0, combT_bf, out_w_bf, ones_bf, outb_row_bf, po_f,
                      out_sb, out_d)
        _epilogue_phase(nc, (2, 3), psB, *ep_args)
        _project_half(nc, 1, combT_bf, out_w_bf, ones_bf, outb_row_bf, po_f,
                      out_sb, out_d)


_CACHE = {}


def build_nc():
    if "nc" in _CACHE:
        return _CACHE["nc"]
    nc = bacc.Bacc(
        "TRN2",
        target_bir_lowering=False,
        debug=False,
        num_devices=N_CORES,
    )
    with tile.TileContext(nc) as tc:
        _build_body(tc)
    nc.compile()
    _CACHE["nc"] = nc
    return nc


def _shared_inputs(inputs):
    f32 = lambda k: np.ascontiguousarray(np.asarray(inputs[k], dtype=np.float32))
    bf = lambda a: np.ascontiguousarray(np.asarray(a, dtype=ml_dtypes.bfloat16))
    attn_b = f32("attn_b").reshape(ATTN)
    dec_b = f32("dec_b").reshape(DEC)
    q = f32("query_w").reshape(DEC)
    # [P, DC] per-partition bias layouts: t[p, dc] = v[dc*128 + p]
    pd = lambda v: np.ascontiguousarray(v.reshape(DC, P).T)
    q_pd = pd(q)
    qz = np.zeros((P, DC, G, G), np.float32)
    for dc in range(DC):
        for j in range(G):
            w = q_pd[:, dc] if j < JT[dc] else -2.0 * q_pd[:, dc]
            qz[:, dc, j, j] = w
    return {
        "attn_w_bf": bf(f32("attn_w")),
        "dec_w_bf": bf(f32("dec_w")),
        "out_w_bf": bf(f32("out_w")),
        "outb_row_bf": bf(f32("out_b").reshape(1, DEC)),
        "ab": pd(attn_b),
        "ab2": pd(2.0 * attn_b),
        "db": pd(dec_b),
        "db2": pd(2.0 * dec_b),
        "qz": bf(qz.reshape(P, DC * G * G)),
    }


def make_core_inputs(inputs, b):
    """Per-core input map for batch b (host-side layout prep only)."""
    bf = lambda a: np.ascontiguousarray(np.asarray(a, dtype=ml_dtypes.bfloat16))
    m = dict(_shared_inputs(inputs))
    ctx = np.asarray(inputs["context"], np.float32)[b]
    out = np.asarray(inputs["output"], np.float32)[b]
    m["ctx_bf"] = bf(ctx)
    m["ctx_t_bf"] = bf(ctx.T)
    m["out_t_bf"] = bf(out.T)
    return m


def kernel(**inputs):
    nc = build_nc()
    in_maps = [make_core_inputs(inputs, b) for b in range(N_CORES)]
    res = bass_utils.run_bass_kernel_spmd(nc, in_maps, core_ids=list(range(N_CORES)))
    _CACHE["last_results"] = res
    out = np.stack([res.results[b]["out"] for b in range(N_CORES)])
    attn = np.stack([res.results[b]["attn"] for b in range(N_CORES)])
    return out, attn


# revision 8
# speedup vs baseline: 5.9725x; 5.9725x over previous
"""Bass/Tile Trainium2 kernel for additive (Bahdanau/'cat') attention.

Problem (per batch b):
  A[i,d]      = sum_a context[i,a] * attn_w[a,d] + attn_b[d]
  O[o,d]      = sum_e output[o,e]  * dec_w[e,d]  + dec_b[d]
  scores[o,i] = sum_d query_w[d] * tanh(A[i,d] + O[o,d])   (+query_b: softmax-invariant)
  attn        = softmax_i(scores)
  mix[o,a]    = sum_i attn[o,i] * context[i,a]
  out[o,d]    = tanh([mix | output] @ out_w + out_b)

Sharding: pure data-parallel over batch, B=8 -> one batch per NeuronCore,
weights broadcast, no collectives.

Per-core structure (v3 — TB/X two-lane):
  * The 16.7M-element nonlinearity is split between ACT and DVE per
    (o, d-chunk) unit with NO broadcast-add anywhere:
    - TB lane (ACT): the raw A^T matmul result stays RESIDENT IN PSUM and
      each unit is ONE activation: Fc[j] = tanh(pa_dc + bias) with
      bias = O^T[d,o] + dec_b + attn_b (biases folded at O^T evacuation).
      PSUM-source activations cost ~(FD+220)/1.2 ns — no DVE involved.
    - X lane (DVE): exp-domain form R' = 1/(1 + e^{2A'} e^{2O'}) computed
      by ONE fused custom DVE op per block (u = Src0*Src1 + 1, bitwise-not
      seed, one Newton pass; in/out bf16; o-batched via stride-0 broadcast
      APs).  tanh(x) = 1 - 2R', so the PE reduction weight for X rows is
      -2*q_d (vs q_d for TB rows); the leftover per-row constant sum q_d
      is softmax-invariant.
    Assignment: dc0/dc1 all-TB (pa resident in 2 banks), dc2 split
    TB rows 0:JT2 + X rows (pa recomputed per phase in a 3rd bank),
    dc3 all-X.  Emission order 0,3,1,2 keeps both engines fed.
  * Weights/context pre-cast to bf16 on the host (layout prep): no
    on-device casts, half the DMA bytes.
  * q-reduction on the PE with zero-padded stationaries: group g of 16 o's
    accumulates into PSUM partitions 32g..32g+15 via tile_position=(0,32g),
    so adjacent groups' matmuls run concurrently in different column
    groups and every evacuation starts at a legal 32-partition boundary.
  * Softmax exp reads scores straight from PSUM (accum_out normalizer);
    attn^T columns are packed per group; mix + final projection run per
    phase-half (o 0..31 / 32..63) so the first half's epilogue and
    projection hide under the second half's main loop.
"""

import numpy as np
import ml_dtypes

import concourse.bass as bass
import concourse.tile as tile
import concourse.bass_utils as bass_utils
from concourse import bacc, mybir, dve_ops
from concourse.dve_ops import DveOp
from concourse.dve_spec import Spec, Src0, Src1, AluOp, Bin, One, C0, C1
from concourse.masks import make_identity

B, OUT_LEN, IN_LEN, DEC, ATTN = 8, 64, 512, 512, 512
P = 128
F32 = mybir.dt.float32
BF16 = mybir.dt.bfloat16
AF = mybir.ActivationFunctionType

G = 16                    # o's per group (= PSUM col-group rows)
NG = OUT_LEN // G         # 4 groups
DC = DEC // P             # 4 d-chunks
AC = ATTN // P            # 4 a-chunks
IC = IN_LEN // P          # 4 i-chunks
EC = DEC // P             # 4 e-chunks
CC = (ATTN + DEC) // P    # 8 combined chunks

N_CORES = 8

# tanh (TB) rows per dc; rows JT..16 go to the DVE exp lane.
JT = (16, 16, 5, 0)

# seed + one-Newton reciprocal constants (Chebyshev pair for the
# [-4.5, -4] interval x*bitcast(~x) lands in; same as RECIP_APPROX_FAST).
C0V, C1V = -0.23549792, 2.0017324

# ---- fused DVE op: out = 1/(Src0*Src1 + 1) ---------------------------------
_u = Src0 * Src1 + One
_nu = Bin(AluOp.BITWISE_NOT, _u, _u)
_y0 = _nu * C0
_RECIP_BODY = _y0 * (C1 - _u * _y0)


def _recip_ref(in0, in1, c0, c1, c2):
    u = (in0.astype(np.float32) * in1.astype(np.float32) + np.float32(1.0)).astype(
        np.float32
    )
    nx = (~u.view(np.int32)).view(np.float32)
    y0 = nx * np.float32(c0)
    return (y0 * (np.float32(c1) - u * y0)).astype(np.float32)


RECIP_AFFINE1 = DveOp(
    "RECIP_AFFINE1_ANT",
    Spec(body=_RECIP_BODY, reference=_recip_ref),
    subdim=False,
    uops_sha={"v3": "4a6026d53837a2bc", "v4": "9de0b962752db8fb"},
)

if RECIP_AFFINE1.name not in dve_ops._SUB_OPCODE_FOR_NAME:
    dve_ops.OPS.append(RECIP_AFFINE1)
    dve_ops.CUSTOM_DVE_SPECS[RECIP_AFFINE1.name] = RECIP_AFFINE1.spec
    dve_ops._SUB_OPCODE_FOR_NAME[RECIP_AFFINE1.name] = (
        dve_ops._CUSTOM_DVE_ROW_BASE + len(dve_ops.OPS) - 1
    )


def _emit_block(nc, g, dc, fpool, pa, OTb2, Pexp, Qexp, QZ, ps, split_x=False):
    """One (group, d-chunk) unit block: 16 o's, one d-chunk of 128.

    pa: resident PSUM A^T (raw, pre-bias) for this dc, or None (all-X dc).
    """
    o0 = G * g
    jt = JT[dc]
    ne = G - jt
    Fc = fpool.tile([P, G, IN_LEN], BF16, tag="F", name=f"F_{g}_{dc}")
    # X rows first (DVE; feeds the PE earliest), then TB singletons on ACT.
    if ne:
        xr = ((jt, G),) if not split_x else ((jt, (jt + G) // 2), ((jt + G) // 2, G))
        for lo, hi in xr:
            n = hi - lo
            nc.vector._custom_dve(
                RECIP_AFFINE1,
                out=Fc[:, lo:hi, :],
                in0=Pexp[:, dc, :].unsqueeze(1).broadcast_to([P, n, IN_LEN]),
                in1=Qexp[:, dc, o0 + lo : o0 + hi].unsqueeze(2).broadcast_to(
                    [P, n, IN_LEN]
                ),
                s0=C0V,
                s1=C1V,
            )
    for j in range(jt):
        nc.scalar.activation(
            Fc[:, j, :], pa[:], AF.Tanh, bias=OTb2[:, dc, o0 + j : o0 + j + 1]
        )
    pv = ps[32 * g : 32 * g + G, :]
    order = list(range(jt, G)) + list(range(jt))
    for k, j in enumerate(order):
        nc.tensor.matmul(
            pv,
            QZ[:, dc, j, :],
            Fc[:, j, :],
            start=(dc == 0 and k == 0),
            stop=(dc == 2 and k == G - 1),   # emission order 0,3,1,2
            tile_position=(0, 32 * g),
            skip_group_check=True,
        )


def _epilogue_phase(nc, gs, ps, ident_bf, exp_sb, sums, recip, attn_sb, attn_bf,
                    attnT_bf, ctx_bf, combT_bf, psum, attn_d):
    """softmax + attn^T + mix for groups gs (one phase half)."""
    for g in gs:
        sl = slice(32 * g, 32 * g + G)
        nc.scalar.activation(exp_sb[sl, :], ps[sl, :], AF.Exp, accum_out=sums[sl, :])
        nc.vector.reciprocal(recip[sl, :], sums[sl, :])
        nc.vector.tensor_scalar_mul(attn_bf[sl, :], exp_sb[sl, :], recip[sl, :])
        nc.vector.tensor_scalar_mul(attn_sb[sl, :], exp_sb[sl, :], recip[sl, :])
        nc.sync.dma_start(attn_d[G * g : G * g + G, :], attn_sb[sl, :])
        for ic in range(IC):
            pt = psum.tile([P, G], BF16, tag="sm", bufs=1, name=f"pt_{g}_{ic}")
            nc.tensor.transpose(
                pt[:], attn_bf[sl, ic * P : (ic + 1) * P], ident_bf[sl, sl],
                tile_position=(32 * g, 0),
            )
            nc.vector.tensor_copy(attnT_bf[:, ic, G * g : G * g + G], pt[:])
    # mix^T for this phase's 32 o-columns
    c0 = G * gs[0]
    cols = slice(c0, c0 + 2 * G)
    for ac in range(AC):
        pm = psum.tile([P, 2 * G], F32, tag="sm", bufs=1, name=f"pm_{gs[0]}_{ac}")
        for ic in range(IC):
            nc.tensor.matmul(
                pm[:],
                ctx_bf[:, ic, ac * P : (ac + 1) * P],
                attnT_bf[:, ic, cols],
                start=(ic == 0),
                stop=(ic == IC - 1),
            )
        nc.vector.tensor_copy(combT_bf[:, ac, cols], pm[:])


def _project_half(nc, ph, combT_bf, out_w_bf, ones_bf, outb_row_bf, po_f, out_sb,
                  out_d):
    """final projection + tanh + store for o rows 32*ph..32*ph+31."""
    hsl = slice(32 * ph, 32 * ph + 32)
    pv = po_f[hsl, :]
    for k in range(CC):
        nc.tensor.matmul(
            pv,
            combT_bf[:, k, hsl],
            out_w_bf[:, k, :],
            start=(k == 0),
            stop=False,
            tile_position=(0, 32 * ph),
        )
    nc.tensor.matmul(
        pv, ones_bf[:, 0:32], outb_row_bf[:], start=False, stop=True,
        tile_position=(0, 32 * ph),
    )
    nc.scalar.activation(out_sb[hsl, :], pv, AF.Tanh)
    nc.sync.dma_start(out_d[hsl, :], out_sb[hsl, :])


def _build_body(tc):
    nc = tc.nc

    # ---- DRAM I/O (per-core shard shapes; weights pre-cast bf16 on host) ----
    ctxT_d = nc.dram_tensor("ctx_t_bf", [ATTN, IN_LEN], BF16, kind="ExternalInput").ap()
    ctx_d = nc.dram_tensor("ctx_bf", [IN_LEN, ATTN], BF16, kind="ExternalInput").ap()
    outT_d = nc.dram_tensor("out_t_bf", [DEC, OUT_LEN], BF16, kind="ExternalInput").ap()
    attn_w_d = nc.dram_tensor("attn_w_bf", [ATTN, DEC], BF16, kind="ExternalInput").ap()
    dec_w_d = nc.dram_tensor("dec_w_bf", [DEC, DEC], BF16, kind="ExternalInput").ap()
    out_w_d = nc.dram_tensor("out_w_bf", [ATTN + DEC, DEC], BF16, kind="ExternalInput").ap()
    outb_row_d = nc.dram_tensor("outb_row_bf", [1, DEC], BF16, kind="ExternalInput").ap()
    ab2_d = nc.dram_tensor("ab2", [P, DC], F32, kind="ExternalInput").ap()
    adb_d = nc.dram_tensor("adb", [P, DC], F32, kind="ExternalInput").ap()
    db2_d = nc.dram_tensor("db2", [P, DC], F32, kind="ExternalInput").ap()
    qz_d = nc.dram_tensor("qz", [P, DC * G * G], BF16, kind="ExternalInput").ap()
    out_d = nc.dram_tensor("out", [OUT_LEN, DEC], F32, kind="ExternalOutput").ap()
    attn_d = nc.dram_tensor("attn", [OUT_LEN, IN_LEN], F32, kind="ExternalOutput").ap()

    from contextlib import ExitStack

    with ExitStack() as ctx:
        const = ctx.enter_context(tc.tile_pool(name="const", bufs=1))
        statics = ctx.enter_context(tc.tile_pool(name="statics", bufs=1))
        fpool = ctx.enter_context(tc.tile_pool(name="fpool", bufs=3))
        psum = ctx.enter_context(tc.tile_pool(name="psum", bufs=1, space="PSUM"))

        # ---------------- constants / identity / warmup ----------------
        ident = const.tile([P, P], F32)
        make_identity(nc, ident)
        ident_bf = const.tile([P, P], BF16)
        nc.vector.tensor_copy(ident_bf[:], ident[:])

        # HAM warmup: real matmul activity flips the PE clock gate to 8/8.
        wu = psum.tile([P, P], F32, tag="mm", bufs=1)
        for _ in range(10):
            nc.tensor.matmul(wu[:], ident_bf[:], ident_bf[:], start=True, stop=True)

        # ---------------- input DMAs ----------------
        ctxT_bf = statics.tile([P, AC, IN_LEN], BF16)    # [a%, ac, i]
        attn_w_bf = statics.tile([P, AC, DEC], BF16)     # [a%, ac, d]
        dec_w_bf = statics.tile([P, EC, DEC], BF16)      # [e%, ec, d]
        ctx_bf = statics.tile([P, IC, ATTN], BF16)       # [i%, ic, a]
        out_w_bf = statics.tile([P, CC, DEC], BF16)      # [c%, cc, d]
        combT_bf = statics.tile([P, CC, OUT_LEN], BF16)  # [c%, cc, o]
        QZ = const.tile([P, DC, G, G], BF16)
        ab2 = const.tile([P, DC], F32)
        adb = const.tile([P, DC], F32)
        db2 = const.tile([P, DC], F32)
        outb_row_bf = const.tile([1, DEC], BF16)
        ones_bf = const.tile([1, OUT_LEN], BF16)
        nc.vector.memset(ones_bf[:], 1.0)

        for ac in range(AC):
            nc.sync.dma_start(ctxT_bf[:, ac, :], ctxT_d[ac * P : (ac + 1) * P, :])
        for ac in range(AC):
            nc.scalar.dma_start(attn_w_bf[:, ac, :], attn_w_d[ac * P : (ac + 1) * P, :])
        for ec in range(EC):
            nc.sync.dma_start(dec_w_bf[:, ec, :], dec_w_d[ec * P : (ec + 1) * P, :])
        for ec in range(EC):
            # output^T chunks double as combined^T chunks 4..7
            nc.scalar.dma_start(combT_bf[:, EC + ec, :], outT_d[ec * P : (ec + 1) * P, :])
        nc.scalar.dma_start(QZ[:], qz_d.rearrange("p (dc j m) -> p dc j m", dc=DC, j=G))
        for t, d in ((ab2, ab2_d), (adb, adb_d), (db2, db2_d)):
            nc.scalar.dma_start(t[:], d)
        nc.scalar.dma_start(outb_row_bf[:], outb_row_d)
        # needed later (mix / final projection): land during phase A
        for ic in range(IC):
            nc.sync.dma_start(ctx_bf[:, ic, :], ctx_d[ic * P : (ic + 1) * P, :])
        for cc in range(CC):
            nc.sync.dma_start(out_w_bf[:, cc, :], out_w_d[cc * P : (cc + 1) * P, :])

        # ---------------- prologue: O^T, A^T, exps ----------------
        OTb2 = statics.tile([P, DC, OUT_LEN], F32)  # O^T + dec_b + attn_b
        Pexp = statics.tile([P, DC, IN_LEN], BF16)  # e^{2(A+attn_b)}
        Qexp = statics.tile([P, DC, OUT_LEN], F32)  # e^{2(O+dec_b)}

        def _ot_dc(dc):
            po = psum.tile([P, OUT_LEN], F32, tag="sm", bufs=1, name=f"po_{dc}")
            for ec in range(EC):
                nc.tensor.matmul(
                    po[:],
                    dec_w_bf[:, ec, dc * P : (dc + 1) * P],
                    combT_bf[:, EC + ec, :],
                    start=(ec == 0),
                    stop=(ec == EC - 1),
                )
            nc.vector.tensor_scalar_add(OTb2[:, dc, :], po[:], adb[:, dc : dc + 1])
            if JT[dc] < G:
                nc.scalar.activation(
                    Qexp[:, dc, :], po[:], AF.Exp, bias=db2[:, dc : dc + 1], scale=2.0
                )

        def _at_dc(dc, pa):
            for ac in range(AC):
                nc.tensor.matmul(
                    pa[:],
                    attn_w_bf[:, ac, dc * P : (dc + 1) * P],
                    ctxT_bf[:, ac, :],
                    start=(ac == 0),
                    stop=(ac == AC - 1),
                )
            if JT[dc] < G:
                nc.scalar.activation(
                    Pexp[:, dc, :], pa[:], AF.Exp, bias=ab2[:, dc : dc + 1], scale=2.0
                )

        # ---------------- epilogue tiles ----------------
        exp_sb = statics.tile([P, IN_LEN], F32)
        sums = statics.tile([P, 1], F32)
        recip = statics.tile([P, 1], F32)
        attn_sb = statics.tile([P, IN_LEN], F32)
        attn_bf = statics.tile([P, IN_LEN], BF16)
        attnT_bf = statics.tile([P, IC, OUT_LEN], BF16)
        out_sb = statics.tile([OUT_LEN, DEC], F32)
        psA = psum.tile([P, IN_LEN], F32, tag="scA", bufs=1, name="psA")
        psB = psum.tile([P, IN_LEN], F32, tag="scB", bufs=1, name="psB")
        po_f = psum.tile([OUT_LEN, DEC], F32, tag="pj", bufs=1, name="po_f")
        # resident raw-A^T psum banks
        pa0 = psum.tile([P, IN_LEN], F32, tag="pa0", bufs=1, name="pa0")
        pa1 = psum.tile([P, IN_LEN], F32, tag="pa1", bufs=1, name="pa1")
        pa2p = ctx.enter_context(tc.tile_pool(name="pa2p", bufs=1, space="PSUM"))

        ep_args = (ident_bf, exp_sb, sums, recip, attn_sb, attn_bf,
                   attnT_bf, ctx_bf, combT_bf, psum, attn_d)

        # ---------------- prologue ----------------
        for dc in range(DC):
            _ot_dc(dc)
        _at_dc(0, pa0)
        pa3 = psum.tile([P, IN_LEN], F32, tag="mm", bufs=1, name="pa3")
        _at_dc(3, pa3)
        _at_dc(1, pa1)

        # ---------------- main loop (dc order 0,3,1,2 per phase) ----------
        PAS = {0: pa0, 1: pa1, 3: None}
        for ph, gs in enumerate(((0, 1), (2, 3))):
            pa2 = pa2p.tile([P, IN_LEN], F32, tag="pa2", name=f"pa2_{ph}")
            _at_dc(2, pa2)
            dcs = (0, 3, 1, 2) if ph == 0 else (3, 1, 2)  # ph1 dc0 pre-emitted
            for dc in dcs:
                pa = pa2 if dc == 2 else PAS[dc]
                ps = psA if ph == 0 else psB
                for g in gs:
                    last = ph == 1 and dc == 2
                    _emit_block(nc, g, dc, fpool, pa, OTb2, Pexp, Qexp, QZ, ps,
                                split_x=last)
            if ph == 0:
                # phase B starts (dc0 blocks) before epilogue A so ACT
                # stays busy while phase A's last matmuls drain
                for g in (2, 3):
                    _emit_block(nc, g, 0, fpool, pa0, OTb2, Pexp, Qexp, QZ, psB)
                _epilogue_phase(nc, (0, 1), psA, *ep_args)
                _project_half(nc, 0, combT_bf, out_w_bf, ones_bf, outb_row_bf,
                              po_f, out_sb, out_d)
        _epilogue_phase(nc, (2, 3), psB, *ep_args)
        _project_half(nc, 1, combT_bf, out_w_bf, ones_bf, outb_row_bf, po_f,
                      out_sb, out_d)


_CACHE = {}


def build_nc():
    if "nc" in _CACHE:
        return _CACHE["nc"]
    nc = bacc.Bacc(
        "TRN2",
        target_bir_lowering=False,
        debug=False,
        num_devices=N_CORES,
    )
    with tile.TileContext(nc) as tc:
        _build_body(tc)
    nc.compile()
    _CACHE["nc"] = nc
    return nc


def _shared_inputs(inputs):
    f32 = lambda k: np.ascontiguousarray(np.asarray(inputs[k], dtype=np.float32))
    bf = lambda a: np.ascontiguousarray(np.asarray(a, dtype=ml_dtypes.bfloat16))
    attn_b = f32("attn_b").reshape(ATTN)
    dec_b = f32("dec_b").reshape(DEC)
    q = f32("query_w").reshape(DEC)
    # [P, DC] per-partition layouts: t[p, dc] = v[dc*128 + p]
    pd = lambda v: np.ascontiguousarray(v.reshape(DC, P).T)
    q_pd = pd(q)
    qz = np.zeros((P, DC, G, G), np.float32)
    for dc in range(DC):
        for j in range(G):
            w = q_pd[:, dc] if j < JT[dc] else -2.0 * q_pd[:, dc]
            qz[:, dc, j, j] = w
    return {
        "attn_w_bf": bf(f32("attn_w")),
        "dec_w_bf": bf(f32("dec_w")),
        "out_w_bf": bf(f32("out_w")),
        "outb_row_bf": bf(f32("out_b").reshape(1, DEC)),
        "ab2": pd(2.0 * attn_b),
        "adb": pd(attn_b + dec_b),
        "db2": pd(2.0 * dec_b),
        "qz": bf(qz.reshape(P, DC * G * G)),
    }


def make_core_inputs(inputs, b):
    """Per-core input map for batch b (host-side layout prep only)."""
    bf = lambda a: np.ascontiguousarray(np.asarray(a, dtype=ml_dtypes.bfloat16))
    m = dict(_shared_inputs(inputs))
    ctx = np.asarray(inputs["context"], np.float32)[b]
    out = np.asarray(inputs["output"], np.float32)[b]
    m["ctx_bf"] = bf(ctx)
    m["ctx_t_bf"] = bf(ctx.T)
    m["out_t_bf"] = bf(out.T)
    return m


def kernel(**inputs):
    nc = build_nc()
    in_maps = [make_core_inputs(inputs, b) for b in range(N_CORES)]
    res = bass_utils.run_bass_kernel_spmd(nc, in_maps, core_ids=list(range(N_CORES)))
    _CACHE["last_results"] = res
    out = np.stack([res.results[b]["out"] for b in range(N_CORES)])
    attn = np.stack([res.results[b]["attn"] for b in range(N_CORES)])
    return out, attn
